# revision 1
# baseline (speedup 1.0000x reference)
import numpy as np

# Per-head sparse MoE (top-2 of 8 experts), expert-parallel across 8 NeuronCores.
# B=8192 tokens, N=16 heads, D=128, H=384, E=8.
# Host: router (replicates reference jnp ops bit-exactly) + token dispatch.
# Device (core e = expert e): per head, h1=w1.T@x, g=wg.T@x, h=h1*silu(g),
# y=w2.T@h on capacity-padded token batches; the per-token routing weight is
# applied on the HOST during unpack (it commutes through the w2 contraction
# along the token axis), so the device never touches it.
# Engine assignment from microbenchmarks: silu 1024-wide on ACT (657ns/op,
# 2x cheaper per elem than 512); h1*sil muls 512-wide on DVE (430ns; 1024-wide
# PSUM-source DVE is pathological at 1608ns); PSUM->SBUF output evacuation
# split 2:3 between DVE (523ns) and ACT (681ns) to balance engine load.
# x/weights/y travel bf16; accumulate f32 in PSUM.

B, N, D, H, E = 8192, 16, 128, 384, 8
N_CORES = 8
HB = H // 128   # h-blocks of 128
WCOL = 3 * H    # packed weight columns per head: [w1 | wg | w2r]

USE_BF16 = True
PSUM_LAYOUT = "g2h2o2"   # or "g2h3o1"

_nc_cache = {}


def _build_bass(C, chunks=None, reps=1, bf16=USE_BF16, layout=None,
                dve_res=(0, 2), sp_bufs=2, hp_bufs=2, pipe=False):
    import concourse.bacc as bacc
    import concourse.mybir as mybir
    import concourse.tile as tile

    layout = layout or PSUM_LAYOUT
    f32 = mybir.dt.float32
    f32r = mybir.dt.float32r
    bf = mybir.dt.bfloat16
    din = bf if bf16 else f32
    dmm = bf if bf16 else f32r
    SILU = mybir.ActivationFunctionType.Silu

    # layout: gW = g tile width, bg/bh/bo = PSUM ring depths; h tiles are
    # [128,512] (1 bank) unless layout starts with a capital H (then 1024).
    cfg = {
        "g2h2o2": (1024, 2, 2, 2, 512),
        "g2h3o1": (1024, 2, 3, 1, 512),
        "g1H2o2": (1024, 1, 2, 2, 1024),
        "G1h2o2": (2048, 1, 2, 2, 512),
    }
    gW, bg, bh, bo, hW = cfg[layout]

    pairs = [(p0, min(gW, C - p0)) for p0 in range(0, C, gW)]

    def subs(pw):
        return [(s, min(512, pw - s)) for s in range(0, pw, 512)]

    nc = bacc.Bacc("TRN2", target_bir_lowering=False, debug=False, num_devices=N_CORES)
    xT = nc.dram_tensor("xT", [N, 128, C], din, kind="ExternalInput").ap()
    wall = nc.dram_tensor("wall", [N, 128, WCOL], din, kind="ExternalInput").ap()
    yT = nc.dram_tensor("yT", [N, 128, C], din, kind="ExternalOutput").ap()

    def cast(ap):
        return ap if bf16 else ap.bitcast(f32r)

    copy_cnt = [0]

    with tile.TileContext(nc) as tc:
        with tc.tile_pool(name="xp", bufs=2) as xp, \
             tc.tile_pool(name="wp", bufs=2) as wp, \
             tc.tile_pool(name="sp", bufs=sp_bufs) as sp, \
             tc.tile_pool(name="hp", bufs=hp_bufs) as hp, \
             tc.tile_pool(name="ob", bufs=4) as ob, \
             tc.tile_pool(name="pg", bufs=bg, space="PSUM") as pg, \
             tc.tile_pool(name="ph", bufs=bh, space="PSUM") as ph, \
             tc.tile_pool(name="po", bufs=bo, space="PSUM") as po:

            pending = [None]

            def emit_o(pend):
                n, p0, pw, hbts, w_t = pend
                w2of = 2 * H
                for (s0, sw) in subs(pw):
                    o_p = po.tile([128, 512], f32, tag="o")
                    for hb in range(HB):
                        nc.tensor.matmul(
                            o_p[:, :sw],
                            w_t[:, w2of + hb * 128:w2of + (hb + 1) * 128],
                            hbts[hb][:, s0:s0 + sw],
                            start=(hb == 0), stop=(hb == HB - 1),
                        )
                    o_sb = ob.tile([128, 512], din, tag="osb")
                    if copy_cnt[0] % 5 in dve_res:
                        nc.vector.tensor_copy(o_sb[:, :sw], o_p[:, :sw])
                    else:
                        nc.scalar.copy(o_sb[:, :sw], o_p[:, :sw])
                    copy_cnt[0] += 1
                    nc.scalar.dma_start(
                        yT[n][:, p0 + s0:p0 + s0 + sw], o_sb[:, :sw]
                    )

            def head(n):
                x_t = xp.tile([128, C], dmm, tag="x")
                nc.sync.dma_start(x_t[:], cast(xT[n]))
                w_t = wp.tile([128, WCOL], dmm, tag="w")
                nc.sync.dma_start(w_t[:], cast(wall[n]))
                w1of, wgof, w2of = 0, H, 2 * H
                for (p0, pw) in pairs:
                    hbts = []
                    for hb in range(HB):
                        g_t = pg.tile([128, gW], f32, tag="g")
                        for (s0, sw) in subs(pw):
                            nc.tensor.matmul(
                                g_t[:, s0:s0 + sw],
                                w_t[:, wgof + hb * 128:wgof + (hb + 1) * 128],
                                x_t[:, p0 + s0:p0 + s0 + sw],
                                start=True, stop=True,
                            )
                        sil = sp.tile([128, gW], f32, tag="sil")
                        nc.scalar.activation(sil[:, :pw], g_t[:, :pw], SILU)
                        hbt = hp.tile([128, gW], dmm, tag=f"hbt{hb}")
                        if hW == 512:
                            for (s0, sw) in subs(pw):
                                h1_t = ph.tile([128, 512], f32, tag="h1")
                                nc.tensor.matmul(
                                    h1_t[:, :sw],
                                    w_t[:, w1of + hb * 128:w1of + (hb + 1) * 128],
                                    x_t[:, p0 + s0:p0 + s0 + sw],
                                    start=True, stop=True,
                                )
                                nc.vector.tensor_mul(
                                    hbt[:, s0:s0 + sw], h1_t[:, :sw],
                                    sil[:, s0:s0 + sw]
                                )
                        else:
                            h1_t = ph.tile([128, hW], f32, tag="h1")
                            for (s0, sw) in subs(pw):
                                nc.tensor.matmul(
                                    h1_t[:, s0:s0 + sw],
                                    w_t[:, w1of + hb * 128:w1of + (hb + 1) * 128],
                                    x_t[:, p0 + s0:p0 + s0 + sw],
                                    start=True, stop=True,
                                )
                            for (s0, sw) in subs(pw):
                                nc.vector.tensor_mul(
                                    hbt[:, s0:s0 + sw], h1_t[:, s0:s0 + sw],
                                    sil[:, s0:s0 + sw]
                                )
                        hbts.append(hbt)
                        if pipe and hb == 0 and pending[0] is not None:
                            emit_o(pending[0])
                            pending[0] = None
                    if pipe:
                        pending[0] = (n, p0, pw, hbts, w_t)
                    else:
                        emit_o((n, p0, pw, hbts, w_t))

            def body():
                for n in range(N):
                    head(n)
                if pipe and pending[0] is not None:
                    emit_o(pending[0])
                    pending[0] = None

            if reps == 1:
                body()
            else:
                with tc.For_i(0, reps, 1):
                    body()
    nc.finalize()
    return nc


def _route(x, router_w):
    import jax
    import jax.numpy as jnp

    router_logits = jnp.asarray(x).reshape(B, N * D) @ jnp.asarray(router_w).T
    topk_logits, topk_idx = jax.lax.top_k(router_logits, 2)
    topk_w = jax.nn.softmax(topk_logits, axis=-1)
    return np.asarray(topk_idx), np.asarray(topk_w).astype(np.float32)


def _dispatch(x, topk_idx, topk_w):
    idx_list, wgt_list = [], []
    for e in range(E):
        sel = np.nonzero((topk_idx == e).any(axis=1))[0]
        we = np.where(topk_idx[sel, 0] == e, topk_w[sel, 0], topk_w[sel, 1])
        idx_list.append(sel)
        wgt_list.append(we.astype(np.float32))
    maxL = max(max(len(s) for s in idx_list), 1)
    C = ((maxL + 127) // 128) * 128
    chunks = []
    c0 = 0
    while c0 < C:
        cw = 512 if C - c0 >= 512 else C - c0
        chunks.append((c0, cw))
        c0 += cw
    return idx_list, wgt_list, C, tuple(chunks)


def _make_in_maps(x, w1, w_gate, w2, idx_list, wgt_list, C, bf16=USE_BF16):
    if bf16:
        import ml_dtypes
        dt = ml_dtypes.bfloat16
    else:
        dt = np.float32
    in_maps = []
    xTfull = np.ascontiguousarray(x.transpose(1, 2, 0).astype(dt))  # (N,128,B)
    for e in range(E):
        sel = idx_list[e]
        L = len(sel)
        xg = np.zeros((N, 128, C), dt)
        if L:
            xg[:, :, :L] = xTfull[:, :, sel]
        w2r = w2[e].reshape(N, HB, 128, 128).transpose(0, 2, 1, 3).reshape(N, 128, H)
        wcat = np.ascontiguousarray(np.concatenate(
            [w1[e].astype(dt), w_gate[e].astype(dt), w2r.astype(dt)], axis=2
        ))  # (N,128,3H)
        in_maps.append({"xT": xg, "wall": wcat})
    return in_maps


_runner_cache = {}


def _make_runner(nc):
    """Cached jitted executor equivalent to bass2jax.run_bass_via_pjrt,
    avoiding per-call retrace/rejit of the shard_map wrapper."""
    import jax
    import concourse.mybir as mybir
    from concourse import bass2jax
    from jax.sharding import Mesh, PartitionSpec
    from jax.experimental.shard_map import shard_map

    bass2jax.install_neuronx_cc_hook()
    partition_name = nc.partition_id_tensor.name if nc.partition_id_tensor else None
    in_names, out_names, out_avals, out_shapes = [], [], [], []
    for alloc in nc.m.functions[0].allocations:
        if not isinstance(alloc, mybir.MemoryLocationSet):
            continue
        name = alloc.memorylocations[0].name
        if alloc.kind == "ExternalInput":
            if name != partition_name:
                in_names.append(name)
        elif alloc.kind == "ExternalOutput":
            shape = tuple(alloc.tensor_shape)
            dtype = mybir.dt.np(alloc.dtype)
            out_names.append(name)
            out_avals.append(jax.core.ShapedArray(shape, dtype))
            out_shapes.append((shape, dtype))
    all_in_names = list(in_names) + list(out_names)
    if partition_name is not None:
        all_in_names.append(partition_name)

    def _body(*args):
        operands = list(args)
        if partition_name is not None:
            operands.append(bass2jax.partition_id_tensor())
        return tuple(bass2jax._bass_exec_p.bind(
            *operands,
            out_avals=tuple(out_avals),
            in_names=tuple(all_in_names),
            out_names=tuple(out_names),
            lowering_input_output_aliases=(),
            sim_require_finite=True,
            sim_require_nnan=True,
            nc=nc,
        ))

    mesh = Mesh(np.asarray(jax.devices()[:N_CORES]), ("core",))
    nio = len(in_names) + len(out_names)
    sharded = jax.jit(
        shard_map(_body, mesh=mesh,
                  in_specs=(PartitionSpec("core"),) * nio,
                  out_specs=(PartitionSpec("core"),) * len(out_names),
                  check_rep=False),
        keep_unused=True,
    )

    def run(in_maps):
        concat_in = [
            np.concatenate([np.asarray(in_maps[c][nm]) for c in range(N_CORES)],
                           axis=0)
            for nm in in_names
        ]
        concat_zeros = [
            np.zeros((N_CORES * s[0], *s[1:]), d) for (s, d) in out_shapes
        ]
        outs = sharded(*(concat_in + concat_zeros))
        outs = [np.asarray(o) for o in outs]
        results = []
        for c in range(N_CORES):
            res = {}
            for (nm, o, (s, d)) in zip(out_names, outs, out_shapes):
                res[nm] = o[c * s[0]:(c + 1) * s[0]]
            results.append(res)
        return results

    return run


def kernel(**inputs):
    x = np.asarray(inputs["x"], dtype=np.float32)
    router_w = np.asarray(inputs["router_w"], dtype=np.float32)
    w1 = np.asarray(inputs["w1"], dtype=np.float32)
    w_gate = np.asarray(inputs["w_gate"], dtype=np.float32)
    w2 = np.asarray(inputs["w2"], dtype=np.float32)

    topk_idx, topk_w = _route(x, router_w)
    idx_list, wgt_list, C, chunks = _dispatch(x, topk_idx, topk_w)

    key = (C, chunks, 1, USE_BF16)
    if key not in _nc_cache:
        _nc_cache[key] = _build_bass(C, chunks)
    nc = _nc_cache[key]

    in_maps = _make_in_maps(x, w1, w_gate, w2, idx_list, wgt_list, C)

    if key not in _runner_cache:
        from concourse import bass_utils
        res = bass_utils.run_bass_kernel_spmd(
            nc, in_maps, core_ids=list(range(N_CORES)), trace=False
        )
        results = res.results
        _runner_cache[key] = _make_runner(nc)
    else:
        results = _runner_cache[key](in_maps)

    out = np.zeros((B, N, D), np.float32)
    for e in range(E):
        sel = idx_list[e]
        L = len(sel)
        if L:
            yT = np.asarray(results[e]["yT"], dtype=np.float32)  # (N,128,C)
            out[sel] += yT[:, :, :L].transpose(2, 0, 1) * \
                wgt_list[e][:, None, None]
    return out



# revision 7
# speedup vs baseline: 1.3402x; 1.3402x over previous
import numpy as np

# Per-head sparse MoE (top-2 of 8 experts), expert-parallel across 8 NeuronCores.
# B=8192 tokens, N=16 heads, D=128, H=384, E=8.
# Host: router (replicates reference jnp ops bit-exactly) + token dispatch.
# Device (core e = expert e): per head, h1=w1.T@x, g=wg.T@x, h=h1*silu(g),
# y=w2.T@h on capacity-padded token batches; the per-token routing weight is
# applied on the HOST during unpack (it commutes through the w2 contraction
# along the token axis), so the device never touches it.
# Engine assignment from microbenchmarks: silu 1024-wide on ACT (657ns/op,
# 2x cheaper per elem than 512); h1*sil muls 512-wide on DVE (430ns; 1024-wide
# PSUM-source DVE is pathological at 1608ns); PSUM->SBUF output evacuation
# split 2:3 between DVE (523ns) and ACT (681ns) to balance engine load.
# x/weights/y travel bf16; accumulate f32 in PSUM.

B, N, D, H, E = 8192, 16, 128, 384, 8
N_CORES = 8
HB = H // 128   # h-blocks of 128
WCOL = 3 * H    # packed weight columns per head: [w1 | wg | w2r]

USE_BF16 = True
PSUM_LAYOUT = "g2h2o2"   # or "g2h3o1"

_nc_cache = {}


def _build_bass(C, chunks=None, reps=1, bf16=USE_BF16, layout=None,
                dve_res=(0, 2), sp_bufs=2, hp_bufs=2, pipe=False):
    import concourse.bacc as bacc
    import concourse.mybir as mybir
    import concourse.tile as tile

    layout = layout or PSUM_LAYOUT
    f32 = mybir.dt.float32
    f32r = mybir.dt.float32r
    bf = mybir.dt.bfloat16
    din = bf if bf16 else f32
    dmm = bf if bf16 else f32r
    SILU = mybir.ActivationFunctionType.Silu

    # layout: gW = g tile width, bg/bh/bo = PSUM ring depths; h tiles are
    # [128,512] (1 bank) unless layout starts with a capital H (then 1024).
    cfg = {
        "g2h2o2": (1024, 2, 2, 2, 512),
        "g2h3o1": (1024, 2, 3, 1, 512),
        "g1H2o2": (1024, 1, 2, 2, 1024),
        "G1h2o2": (2048, 1, 2, 2, 512),
    }
    gW, bg, bh, bo, hW = cfg[layout]

    pairs = [(p0, min(gW, C - p0)) for p0 in range(0, C, gW)]

    def subs(pw):
        return [(s, min(512, pw - s)) for s in range(0, pw, 512)]

    nc = bacc.Bacc("TRN2", target_bir_lowering=False, debug=False, num_devices=N_CORES)
    xT = nc.dram_tensor("xT", [N, 128, C], din, kind="ExternalInput").ap()
    wall = nc.dram_tensor("wall", [N, 128, WCOL], din, kind="ExternalInput").ap()
    yT = nc.dram_tensor("yT", [N, 128, C], din, kind="ExternalOutput").ap()

    def cast(ap):
        return ap if bf16 else ap.bitcast(f32r)

    copy_cnt = [0]

    with tile.TileContext(nc) as tc:
        with tc.tile_pool(name="xp", bufs=2) as xp, \
             tc.tile_pool(name="wp", bufs=2) as wp, \
             tc.tile_pool(name="sp", bufs=sp_bufs) as sp, \
             tc.tile_pool(name="hp", bufs=hp_bufs) as hp, \
             tc.tile_pool(name="ob", bufs=4) as ob, \
             tc.tile_pool(name="pg", bufs=bg, space="PSUM") as pg, \
             tc.tile_pool(name="ph", bufs=bh, space="PSUM") as ph, \
             tc.tile_pool(name="po", bufs=bo, space="PSUM") as po:

            pending = [None]

            def emit_o(pend):
                n, p0, pw, hbts, w_t = pend
                w2of = 2 * H
                for (s0, sw) in subs(pw):
                    o_p = po.tile([128, 512], f32, tag="o")
                    for hb in range(HB):
                        nc.tensor.matmul(
                            o_p[:, :sw],
                            w_t[:, w2of + hb * 128:w2of + (hb + 1) * 128],
                            hbts[hb][:, s0:s0 + sw],
                            start=(hb == 0), stop=(hb == HB - 1),
                        )
                    o_sb = ob.tile([128, 512], din, tag="osb")
                    if copy_cnt[0] % 5 in dve_res:
                        nc.vector.tensor_copy(o_sb[:, :sw], o_p[:, :sw])
                    else:
                        nc.scalar.copy(o_sb[:, :sw], o_p[:, :sw])
                    copy_cnt[0] += 1
                    nc.scalar.dma_start(
                        yT[n][:, p0 + s0:p0 + s0 + sw], o_sb[:, :sw]
                    )

            def head(n):
                x_t = xp.tile([128, C], dmm, tag="x")
                nc.sync.dma_start(x_t[:], cast(xT[n]))
                w_t = wp.tile([128, WCOL], dmm, tag="w")
                nc.sync.dma_start(w_t[:], cast(wall[n]))
                w1of, wgof, w2of = 0, H, 2 * H
                for (p0, pw) in pairs:
                    hbts = []
                    for hb in range(HB):
                        g_t = pg.tile([128, gW], f32, tag="g")
                        for (s0, sw) in subs(pw):
                            nc.tensor.matmul(
                                g_t[:, s0:s0 + sw],
                                w_t[:, wgof + hb * 128:wgof + (hb + 1) * 128],
                                x_t[:, p0 + s0:p0 + s0 + sw],
                                start=True, stop=True,
                            )
                        sil = sp.tile([128, gW], f32, tag="sil")
                        nc.scalar.activation(sil[:, :pw], g_t[:, :pw], SILU)
                        hbt = hp.tile([128, gW], dmm, tag=f"hbt{hb}")
                        if hW == 512:
                            for (s0, sw) in subs(pw):
                                h1_t = ph.tile([128, 512], f32, tag="h1")
                                nc.tensor.matmul(
                                    h1_t[:, :sw],
                                    w_t[:, w1of + hb * 128:w1of + (hb + 1) * 128],
                                    x_t[:, p0 + s0:p0 + s0 + sw],
                                    start=True, stop=True,
                                )
                                nc.vector.tensor_mul(
                                    hbt[:, s0:s0 + sw], h1_t[:, :sw],
                                    sil[:, s0:s0 + sw]
                                )
                        else:
                            h1_t = ph.tile([128, hW], f32, tag="h1")
                            for (s0, sw) in subs(pw):
                                nc.tensor.matmul(
                                    h1_t[:, s0:s0 + sw],
                                    w_t[:, w1of + hb * 128:w1of + (hb + 1) * 128],
                                    x_t[:, p0 + s0:p0 + s0 + sw],
                                    start=True, stop=True,
                                )
                            for (s0, sw) in subs(pw):
                                nc.vector.tensor_mul(
                                    hbt[:, s0:s0 + sw], h1_t[:, s0:s0 + sw],
                                    sil[:, s0:s0 + sw]
                                )
                        hbts.append(hbt)
                        if pipe and hb == 0 and pending[0] is not None:
                            emit_o(pending[0])
                            pending[0] = None
                    if pipe:
                        pending[0] = (n, p0, pw, hbts, w_t)
                    else:
                        emit_o((n, p0, pw, hbts, w_t))

            def body():
                for n in range(N):
                    head(n)
                if pipe and pending[0] is not None:
                    emit_o(pending[0])
                    pending[0] = None

            if reps == 1:
                body()
            else:
                with tc.For_i(0, reps, 1):
                    body()
    nc.finalize()
    return nc


def _route(x, router_w):
    import jax
    import jax.numpy as jnp

    router_logits = jnp.asarray(x).reshape(B, N * D) @ jnp.asarray(router_w).T
    topk_logits, topk_idx = jax.lax.top_k(router_logits, 2)
    topk_w = jax.nn.softmax(topk_logits, axis=-1)
    return np.asarray(topk_idx), np.asarray(topk_w).astype(np.float32)


def _dispatch(x, topk_idx, topk_w):
    idx_list, wgt_list = [], []
    for e in range(E):
        sel = np.nonzero((topk_idx == e).any(axis=1))[0]
        we = np.where(topk_idx[sel, 0] == e, topk_w[sel, 0], topk_w[sel, 1])
        idx_list.append(sel)
        wgt_list.append(we.astype(np.float32))
    maxL = max(max(len(s) for s in idx_list), 1)
    C = ((maxL + 127) // 128) * 128
    chunks = []
    c0 = 0
    while c0 < C:
        cw = 512 if C - c0 >= 512 else C - c0
        chunks.append((c0, cw))
        c0 += cw
    return idx_list, wgt_list, C, tuple(chunks)


def _make_in_maps(x, w1, w_gate, w2, idx_list, wgt_list, C, bf16=USE_BF16):
    if bf16:
        import ml_dtypes
        dt = ml_dtypes.bfloat16
    else:
        dt = np.float32
    in_maps = []
    xTfull = np.ascontiguousarray(x.transpose(1, 2, 0).astype(dt))  # (N,128,B)
    for e in range(E):
        sel = idx_list[e]
        L = len(sel)
        xg = np.zeros((N, 128, C), dt)
        if L:
            xg[:, :, :L] = xTfull[:, :, sel]
        w2r = w2[e].reshape(N, HB, 128, 128).transpose(0, 2, 1, 3).reshape(N, 128, H)
        wcat = np.ascontiguousarray(np.concatenate(
            [w1[e].astype(dt), w_gate[e].astype(dt), w2r.astype(dt)], axis=2
        ))  # (N,128,3H)
        in_maps.append({"xT": xg, "wall": wcat})
    return in_maps


_runner_cache = {}


def _make_runner(nc):
    """Cached jitted executor equivalent to bass2jax.run_bass_via_pjrt,
    avoiding per-call retrace/rejit of the shard_map wrapper."""
    import jax
    import concourse.mybir as mybir
    from concourse import bass2jax
    from jax.sharding import Mesh, PartitionSpec
    from jax.experimental.shard_map import shard_map

    bass2jax.install_neuronx_cc_hook()
    partition_name = nc.partition_id_tensor.name if nc.partition_id_tensor else None
    in_names, out_names, out_avals, out_shapes = [], [], [], []
    for alloc in nc.m.functions[0].allocations:
        if not isinstance(alloc, mybir.MemoryLocationSet):
            continue
        name = alloc.memorylocations[0].name
        if alloc.kind == "ExternalInput":
            if name != partition_name:
                in_names.append(name)
        elif alloc.kind == "ExternalOutput":
            shape = tuple(alloc.tensor_shape)
            dtype = mybir.dt.np(alloc.dtype)
            out_names.append(name)
            out_avals.append(jax.core.ShapedArray(shape, dtype))
            out_shapes.append((shape, dtype))
    all_in_names = list(in_names) + list(out_names)
    if partition_name is not None:
        all_in_names.append(partition_name)

    def _body(*args):
        operands = list(args)
        if partition_name is not None:
            operands.append(bass2jax.partition_id_tensor())
        return tuple(bass2jax._bass_exec_p.bind(
            *operands,
            out_avals=tuple(out_avals),
            in_names=tuple(all_in_names),
            out_names=tuple(out_names),
            lowering_input_output_aliases=(),
            sim_require_finite=True,
            sim_require_nnan=True,
            nc=nc,
        ))

    mesh = Mesh(np.asarray(jax.devices()[:N_CORES]), ("core",))
    nio = len(in_names) + len(out_names)
    sharded = jax.jit(
        shard_map(_body, mesh=mesh,
                  in_specs=(PartitionSpec("core"),) * nio,
                  out_specs=(PartitionSpec("core"),) * len(out_names),
                  check_rep=False),
        keep_unused=True,
    )

    def run(in_maps):
        concat_in = [
            np.concatenate([np.asarray(in_maps[c][nm]) for c in range(N_CORES)],
                           axis=0)
            for nm in in_names
        ]
        concat_zeros = [
            np.zeros((N_CORES * s[0], *s[1:]), d) for (s, d) in out_shapes
        ]
        outs = sharded(*(concat_in + concat_zeros))
        outs = [np.asarray(o) for o in outs]
        results = []
        for c in range(N_CORES):
            res = {}
            for (nm, o, (s, d)) in zip(out_names, outs, out_shapes):
                res[nm] = o[c * s[0]:(c + 1) * s[0]]
            results.append(res)
        return results

    return run


def kernel(**inputs):
    x = np.asarray(inputs["x"], dtype=np.float32)
    router_w = np.asarray(inputs["router_w"], dtype=np.float32)
    w1 = np.asarray(inputs["w1"], dtype=np.float32)
    w_gate = np.asarray(inputs["w_gate"], dtype=np.float32)
    w2 = np.asarray(inputs["w2"], dtype=np.float32)

    topk_idx, topk_w = _route(x, router_w)
    idx_list, wgt_list, C, chunks = _dispatch(x, topk_idx, topk_w)

    key = (C, chunks, 1, USE_BF16)
    if key not in _nc_cache:
        _nc_cache[key] = _build_bass(C, chunks)
    nc = _nc_cache[key]

    in_maps = _make_in_maps(x, w1, w_gate, w2, idx_list, wgt_list, C)

    if key not in _runner_cache:
        from concourse import bass_utils
        res = bass_utils.run_bass_kernel_spmd(
            nc, in_maps, core_ids=list(range(N_CORES)), trace=False
        )
        results = res.results
        _runner_cache[key] = _make_runner(nc)
    else:
        results = _runner_cache[key](in_maps)

    out = np.zeros((B, N, D), np.float32)
    for e in range(E):
        sel = idx_list[e]
        L = len(sel)
        if L:
            yT = np.asarray(results[e]["yT"], dtype=np.float32)  # (N,128,C)
            out[sel] += yT[:, :, :L].transpose(2, 0, 1) * \
                wgt_list[e][:, None, None]
    return out



# revision 10
# speedup vs baseline: 1.3563x; 1.0121x over previous
import numpy as np

# Per-head sparse MoE (top-2 of 8 experts), expert-parallel across 8 NeuronCores.
# B=8192 tokens, N=16 heads, D=128, H=384, E=8.
# Host: router (replicates reference jnp ops bit-exactly) + token dispatch.
# Device (core e = expert e): per head, h1=w1.T@x, g=wg.T@x, h=h1*silu(g),
# y=w2.T@h on capacity-padded token batches; the per-token routing weight is
# applied on the HOST during unpack (it commutes through the w2 contraction
# along the token axis), so the device never touches it.
# Engine assignment from microbenchmarks: silu 1024-wide on ACT (657ns/op,
# 2x cheaper per elem than 512); h1*sil muls 512-wide on DVE (430ns; 1024-wide
# PSUM-source DVE is pathological at 1608ns); PSUM->SBUF output evacuation
# split 2:3 between DVE (523ns) and ACT (681ns) to balance engine load.
# x/weights/y travel bf16; accumulate f32 in PSUM.

B, N, D, H, E = 8192, 16, 128, 384, 8
N_CORES = 8
HB = H // 128   # h-blocks of 128
WCOL = 3 * H    # packed weight columns per head: [w1 | wg | w2r]

USE_BF16 = True
PSUM_LAYOUT = "g2h2o2"   # or "g2h3o1"

_nc_cache = {}


def _build_bass(C, chunks=None, reps=1, bf16=USE_BF16, layout=None,
                dve_res=(0, 2), sp_bufs=2, hp_bufs=2, pipe=False):
    import concourse.bacc as bacc
    import concourse.mybir as mybir
    import concourse.tile as tile

    layout = layout or PSUM_LAYOUT
    f32 = mybir.dt.float32
    f32r = mybir.dt.float32r
    bf = mybir.dt.bfloat16
    din = bf if bf16 else f32
    dmm = bf if bf16 else f32r
    SILU = mybir.ActivationFunctionType.Silu

    # layout: gW = g tile width, bg/bh/bo = PSUM ring depths; h tiles are
    # [128,512] (1 bank) unless layout starts with a capital H (then 1024).
    cfg = {
        "g2h2o2": (1024, 2, 2, 2, 512),
        "g2h3o1": (1024, 2, 3, 1, 512),
        "g1H2o2": (1024, 1, 2, 2, 1024),
        "G1h2o2": (2048, 1, 2, 2, 512),
    }
    gW, bg, bh, bo, hW = cfg[layout]

    pairs = [(p0, min(gW, C - p0)) for p0 in range(0, C, gW)]

    def subs(pw):
        return [(s, min(512, pw - s)) for s in range(0, pw, 512)]

    nc = bacc.Bacc("TRN2", target_bir_lowering=False, debug=False, num_devices=N_CORES)
    xT = nc.dram_tensor("xT", [N, 128, C], din, kind="ExternalInput").ap()
    wall = nc.dram_tensor("wall", [N, 128, WCOL], din, kind="ExternalInput").ap()
    yT = nc.dram_tensor("yT", [N, 128, C], din, kind="ExternalOutput").ap()

    def cast(ap):
        return ap if bf16 else ap.bitcast(f32r)

    copy_cnt = [0]

    with tile.TileContext(nc) as tc:
        with tc.tile_pool(name="xp", bufs=2) as xp, \
             tc.tile_pool(name="wp", bufs=2) as wp, \
             tc.tile_pool(name="sp", bufs=sp_bufs) as sp, \
             tc.tile_pool(name="hp", bufs=hp_bufs) as hp, \
             tc.tile_pool(name="ob", bufs=4) as ob, \
             tc.tile_pool(name="pg", bufs=bg, space="PSUM") as pg, \
             tc.tile_pool(name="ph", bufs=bh, space="PSUM") as ph, \
             tc.tile_pool(name="po", bufs=bo, space="PSUM") as po:

            pending = [None]

            def emit_o(pend):
                n, p0, pw, hbts, w_t = pend
                w2of = 2 * H
                for (s0, sw) in subs(pw):
                    o_p = po.tile([128, 512], f32, tag="o")
                    for hb in range(HB):
                        nc.tensor.matmul(
                            o_p[:, :sw],
                            w_t[:, w2of + hb * 128:w2of + (hb + 1) * 128],
                            hbts[hb][:, s0:s0 + sw],
                            start=(hb == 0), stop=(hb == HB - 1),
                        )
                    o_sb = ob.tile([128, 512], din, tag="osb")
                    if copy_cnt[0] % 5 in dve_res:
                        nc.vector.tensor_copy(o_sb[:, :sw], o_p[:, :sw])
                    else:
                        nc.scalar.copy(o_sb[:, :sw], o_p[:, :sw])
                    copy_cnt[0] += 1
                    nc.scalar.dma_start(
                        yT[n][:, p0 + s0:p0 + s0 + sw], o_sb[:, :sw]
                    )

            def head(n):
                x_t = xp.tile([128, C], dmm, tag="x")
                nc.sync.dma_start(x_t[:], cast(xT[n]))
                w_t = wp.tile([128, WCOL], dmm, tag="w")
                nc.sync.dma_start(w_t[:], cast(wall[n]))
                w1of, wgof, w2of = 0, H, 2 * H
                for (p0, pw) in pairs:
                    hbts = []
                    for hb in range(HB):
                        g_t = pg.tile([128, gW], f32, tag="g")
                        for (s0, sw) in subs(pw):
                            nc.tensor.matmul(
                                g_t[:, s0:s0 + sw],
                                w_t[:, wgof + hb * 128:wgof + (hb + 1) * 128],
                                x_t[:, p0 + s0:p0 + s0 + sw],
                                start=True, stop=True,
                            )
                        sil = sp.tile([128, gW], f32, tag="sil")
                        nc.scalar.activation(sil[:, :pw], g_t[:, :pw], SILU)
                        hbt = hp.tile([128, gW], dmm, tag=f"hbt{hb}")
                        if hW == 512:
                            for (s0, sw) in subs(pw):
                                h1_t = ph.tile([128, 512], f32, tag="h1")
                                nc.tensor.matmul(
                                    h1_t[:, :sw],
                                    w_t[:, w1of + hb * 128:w1of + (hb + 1) * 128],
                                    x_t[:, p0 + s0:p0 + s0 + sw],
                                    start=True, stop=True,
                                )
                                nc.vector.tensor_mul(
                                    hbt[:, s0:s0 + sw], h1_t[:, :sw],
                                    sil[:, s0:s0 + sw]
                                )
                        else:
                            h1_t = ph.tile([128, hW], f32, tag="h1")
                            for (s0, sw) in subs(pw):
                                nc.tensor.matmul(
                                    h1_t[:, s0:s0 + sw],
                                    w_t[:, w1of + hb * 128:w1of + (hb + 1) * 128],
                                    x_t[:, p0 + s0:p0 + s0 + sw],
                                    start=True, stop=True,
                                )
                            for (s0, sw) in subs(pw):
                                nc.vector.tensor_mul(
                                    hbt[:, s0:s0 + sw], h1_t[:, s0:s0 + sw],
                                    sil[:, s0:s0 + sw]
                                )
                        hbts.append(hbt)
                        if pipe and hb == 0 and pending[0] is not None:
                            emit_o(pending[0])
                            pending[0] = None
                    if pipe:
                        pending[0] = (n, p0, pw, hbts, w_t)
                    else:
                        emit_o((n, p0, pw, hbts, w_t))

            def body():
                for n in range(N):
                    head(n)
                if pipe and pending[0] is not None:
                    emit_o(pending[0])
                    pending[0] = None

            if reps == 1:
                body()
            else:
                with tc.For_i(0, reps, 1):
                    body()
    nc.finalize()
    return nc


def _route(x, router_w):
    import jax
    import jax.numpy as jnp

    router_logits = jnp.asarray(x).reshape(B, N * D) @ jnp.asarray(router_w).T
    topk_logits, topk_idx = jax.lax.top_k(router_logits, 2)
    topk_w = jax.nn.softmax(topk_logits, axis=-1)
    return np.asarray(topk_idx), np.asarray(topk_w).astype(np.float32)


def _dispatch(x, topk_idx, topk_w):
    idx_list, wgt_list = [], []
    for e in range(E):
        sel = np.nonzero((topk_idx == e).any(axis=1))[0]
        we = np.where(topk_idx[sel, 0] == e, topk_w[sel, 0], topk_w[sel, 1])
        idx_list.append(sel)
        wgt_list.append(we.astype(np.float32))
    maxL = max(max(len(s) for s in idx_list), 1)
    C = ((maxL + 127) // 128) * 128
    chunks = []
    c0 = 0
    while c0 < C:
        cw = 512 if C - c0 >= 512 else C - c0
        chunks.append((c0, cw))
        c0 += cw
    return idx_list, wgt_list, C, tuple(chunks)


def _make_in_maps(x, w1, w_gate, w2, idx_list, wgt_list, C, bf16=USE_BF16):
    if bf16:
        import ml_dtypes
        dt = ml_dtypes.bfloat16
    else:
        dt = np.float32
    in_maps = []
    xTfull = np.ascontiguousarray(x.transpose(1, 2, 0).astype(dt))  # (N,128,B)
    for e in range(E):
        sel = idx_list[e]
        L = len(sel)
        xg = np.zeros((N, 128, C), dt)
        if L:
            xg[:, :, :L] = xTfull[:, :, sel]
        w2r = w2[e].reshape(N, HB, 128, 128).transpose(0, 2, 1, 3).reshape(N, 128, H)
        wcat = np.ascontiguousarray(np.concatenate(
            [w1[e].astype(dt), w_gate[e].astype(dt), w2r.astype(dt)], axis=2
        ))  # (N,128,3H)
        in_maps.append({"xT": xg, "wall": wcat})
    return in_maps


_runner_cache = {}


def _make_runner(nc):
    """Cached jitted executor equivalent to bass2jax.run_bass_via_pjrt,
    avoiding per-call retrace/rejit of the shard_map wrapper."""
    import jax
    import concourse.mybir as mybir
    from concourse import bass2jax
    from jax.sharding import Mesh, PartitionSpec
    from jax.experimental.shard_map import shard_map

    bass2jax.install_neuronx_cc_hook()
    partition_name = nc.partition_id_tensor.name if nc.partition_id_tensor else None
    in_names, out_names, out_avals, out_shapes = [], [], [], []
    for alloc in nc.m.functions[0].allocations:
        if not isinstance(alloc, mybir.MemoryLocationSet):
            continue
        name = alloc.memorylocations[0].name
        if alloc.kind == "ExternalInput":
            if name != partition_name:
                in_names.append(name)
        elif alloc.kind == "ExternalOutput":
            shape = tuple(alloc.tensor_shape)
            dtype = mybir.dt.np(alloc.dtype)
            out_names.append(name)
            out_avals.append(jax.core.ShapedArray(shape, dtype))
            out_shapes.append((shape, dtype))
    all_in_names = list(in_names) + list(out_names)
    if partition_name is not None:
        all_in_names.append(partition_name)

    def _body(*args):
        operands = list(args)
        if partition_name is not None:
            operands.append(bass2jax.partition_id_tensor())
        return tuple(bass2jax._bass_exec_p.bind(
            *operands,
            out_avals=tuple(out_avals),
            in_names=tuple(all_in_names),
            out_names=tuple(out_names),
            lowering_input_output_aliases=(),
            sim_require_finite=True,
            sim_require_nnan=True,
            nc=nc,
        ))

    mesh = Mesh(np.asarray(jax.devices()[:N_CORES]), ("core",))
    nio = len(in_names) + len(out_names)
    sharded = jax.jit(
        shard_map(_body, mesh=mesh,
                  in_specs=(PartitionSpec("core"),) * nio,
                  out_specs=(PartitionSpec("core"),) * len(out_names),
                  check_rep=False),
        keep_unused=True,
    )

    def run(in_maps):
        concat_in = [
            np.concatenate([np.asarray(in_maps[c][nm]) for c in range(N_CORES)],
                           axis=0)
            for nm in in_names
        ]
        concat_zeros = [
            np.zeros((N_CORES * s[0], *s[1:]), d) for (s, d) in out_shapes
        ]
        outs = sharded(*(concat_in + concat_zeros))
        outs = [np.asarray(o) for o in outs]
        results = []
        for c in range(N_CORES):
            res = {}
            for (nm, o, (s, d)) in zip(out_names, outs, out_shapes):
                res[nm] = o[c * s[0]:(c + 1) * s[0]]
            results.append(res)
        return results

    return run


def kernel(**inputs):
    x = np.asarray(inputs["x"], dtype=np.float32)
    router_w = np.asarray(inputs["router_w"], dtype=np.float32)
    w1 = np.asarray(inputs["w1"], dtype=np.float32)
    w_gate = np.asarray(inputs["w_gate"], dtype=np.float32)
    w2 = np.asarray(inputs["w2"], dtype=np.float32)

    topk_idx, topk_w = _route(x, router_w)
    idx_list, wgt_list, C, chunks = _dispatch(x, topk_idx, topk_w)

    key = (C, chunks, 1, USE_BF16)
    if key not in _nc_cache:
        _nc_cache[key] = _build_bass(C, chunks)
    nc = _nc_cache[key]

    in_maps = _make_in_maps(x, w1, w_gate, w2, idx_list, wgt_list, C)

    if key not in _runner_cache:
        from concourse import bass_utils
        res = bass_utils.run_bass_kernel_spmd(
            nc, in_maps, core_ids=list(range(N_CORES)), trace=False
        )
        results = res.results
        _runner_cache[key] = _make_runner(nc)
    else:
        results = _runner_cache[key](in_maps)

    out = np.zeros((B, N, D), np.float32)
    for e in range(E):
        sel = idx_list[e]
        L = len(sel)
        if L:
            yT = np.asarray(results[e]["yT"], dtype=np.float32)  # (N,128,C)
            out[sel] += yT[:, :, :L].transpose(2, 0, 1) * \
                wgt_list[e][:, None, None]
    return out



# revision 13
# speedup vs baseline: 1.3641x; 1.0057x over previous
import numpy as np

# Per-head sparse MoE (top-2 of 8 experts), expert-parallel across 8 NeuronCores.
# B=8192 tokens, N=16 heads, D=128, H=384, E=8.
# Host: router (replicates reference jnp ops bit-exactly) + token dispatch.
# Device (core e = expert e): per head, h1=w1.T@x, g=wg.T@x, h=h1*silu(g),
# y=w2.T@h on capacity-padded token batches; the per-token routing weight is
# applied on the HOST during unpack (it commutes through the w2 contraction
# along the token axis), so the device never touches it.
# Engine assignment from microbenchmarks: silu 1024-wide on ACT (657ns/op,
# 2x cheaper per elem than 512); h1*sil muls 512-wide on DVE (430ns; 1024-wide
# PSUM-source DVE is pathological at 1608ns); PSUM->SBUF output evacuation
# split 2:3 between DVE (523ns) and ACT (681ns) to balance engine load.
# y-DMA descriptors are issued from the idle GpSimd queue (ydma_gp): keeping
# the 68 DMA-trigger instructions out of ACT's queue measures ~8us faster.
# x/weights/y travel bf16; accumulate f32 in PSUM.

B, N, D, H, E = 8192, 16, 128, 384, 8
N_CORES = 8
HB = H // 128   # h-blocks of 128
WCOL = 3 * H    # packed weight columns per head: [w1 | wg | w2r]

USE_BF16 = True
PSUM_LAYOUT = "g2h2o2"   # or "g2h3o1"

_nc_cache = {}


def _build_bass(C, chunks=None, reps=1, bf16=USE_BF16, layout=None,
                dve_res=(0, 2), sp_bufs=2, hp_bufs=2, pipe=False,
                mul_swap=False, ydma_gp=True, y_f32=False):
    import concourse.bacc as bacc
    import concourse.mybir as mybir
    import concourse.tile as tile

    layout = layout or PSUM_LAYOUT
    f32 = mybir.dt.float32
    f32r = mybir.dt.float32r
    bf = mybir.dt.bfloat16
    din = bf if bf16 else f32
    dmm = bf if bf16 else f32r
    SILU = mybir.ActivationFunctionType.Silu

    # layout: gW = g tile width, bg/bh/bo = PSUM ring depths; h tiles are
    # [128,512] (1 bank) unless layout starts with a capital H (then 1024).
    cfg = {
        "g2h2o2": (1024, 2, 2, 2, 512),
        "g2h3o1": (1024, 2, 3, 1, 512),
        "g1H2o2": (1024, 1, 2, 2, 1024),
        "G1h2o2": (2048, 1, 2, 2, 512),
    }
    gW, bg, bh, bo, hW = cfg[layout]

    pairs = [(p0, min(gW, C - p0)) for p0 in range(0, C, gW)]

    def subs(pw):
        return [(s, min(512, pw - s)) for s in range(0, pw, 512)]

    nc = bacc.Bacc("TRN2", target_bir_lowering=False, debug=False, num_devices=N_CORES)
    xT = nc.dram_tensor("xT", [N, 128, C], din, kind="ExternalInput").ap()
    wall = nc.dram_tensor("wall", [N, 128, WCOL], din, kind="ExternalInput").ap()
    dout = f32 if y_f32 else din
    yT = nc.dram_tensor("yT", [N, 128, C], dout, kind="ExternalOutput").ap()

    def cast(ap):
        return ap if bf16 else ap.bitcast(f32r)

    copy_cnt = [0]

    with tile.TileContext(nc) as tc:
        with tc.tile_pool(name="xp", bufs=2) as xp, \
             tc.tile_pool(name="wp", bufs=2) as wp, \
             tc.tile_pool(name="sp", bufs=sp_bufs) as sp, \
             tc.tile_pool(name="hp", bufs=hp_bufs) as hp, \
             tc.tile_pool(name="ob", bufs=4) as ob, \
             tc.tile_pool(name="pg", bufs=bg, space="PSUM") as pg, \
             tc.tile_pool(name="ph", bufs=bh, space="PSUM") as ph, \
             tc.tile_pool(name="po", bufs=bo, space="PSUM") as po:

            pending = [None]

            def emit_o(pend):
                n, p0, pw, hbts, w_t = pend
                w2of = 2 * H
                for (s0, sw) in subs(pw):
                    o_p = po.tile([128, 512], f32, tag="o")
                    for hb in range(HB):
                        nc.tensor.matmul(
                            o_p[:, :sw],
                            w_t[:, w2of + hb * 128:w2of + (hb + 1) * 128],
                            hbts[hb][:, s0:s0 + sw],
                            start=(hb == 0), stop=(hb == HB - 1),
                        )
                    o_sb = ob.tile([128, 512], dout, tag="osb")
                    if copy_cnt[0] % 5 in dve_res:
                        nc.vector.tensor_copy(o_sb[:, :sw], o_p[:, :sw])
                    else:
                        nc.scalar.copy(o_sb[:, :sw], o_p[:, :sw])
                    copy_cnt[0] += 1
                    (nc.gpsimd if ydma_gp else nc.scalar).dma_start(
                        yT[n][:, p0 + s0:p0 + s0 + sw], o_sb[:, :sw]
                    )

            def head(n):
                x_t = xp.tile([128, C], dmm, tag="x")
                nc.sync.dma_start(x_t[:], cast(xT[n]))
                w_t = wp.tile([128, WCOL], dmm, tag="w")
                nc.sync.dma_start(w_t[:], cast(wall[n]))
                w1of, wgof, w2of = 0, H, 2 * H
                for (p0, pw) in pairs:
                    hbts = []
                    for hb in range(HB):
                        g_t = pg.tile([128, gW], f32, tag="g")
                        for (s0, sw) in subs(pw):
                            nc.tensor.matmul(
                                g_t[:, s0:s0 + sw],
                                w_t[:, wgof + hb * 128:wgof + (hb + 1) * 128],
                                x_t[:, p0 + s0:p0 + s0 + sw],
                                start=True, stop=True,
                            )
                        sil = sp.tile([128, gW], f32, tag="sil")
                        nc.scalar.activation(sil[:, :pw], g_t[:, :pw], SILU)
                        hbt = hp.tile([128, gW], dmm, tag=f"hbt{hb}")
                        if hW == 512:
                            for (s0, sw) in subs(pw):
                                h1_t = ph.tile([128, 512], f32, tag="h1")
                                nc.tensor.matmul(
                                    h1_t[:, :sw],
                                    w_t[:, w1of + hb * 128:w1of + (hb + 1) * 128],
                                    x_t[:, p0 + s0:p0 + s0 + sw],
                                    start=True, stop=True,
                                )
                                if mul_swap:
                                    nc.vector.tensor_mul(
                                        hbt[:, s0:s0 + sw],
                                        sil[:, s0:s0 + sw], h1_t[:, :sw]
                                    )
                                else:
                                    nc.vector.tensor_mul(
                                        hbt[:, s0:s0 + sw], h1_t[:, :sw],
                                        sil[:, s0:s0 + sw]
                                    )
                        else:
                            h1_t = ph.tile([128, hW], f32, tag="h1")
                            for (s0, sw) in subs(pw):
                                nc.tensor.matmul(
                                    h1_t[:, s0:s0 + sw],
                                    w_t[:, w1of + hb * 128:w1of + (hb + 1) * 128],
                                    x_t[:, p0 + s0:p0 + s0 + sw],
                                    start=True, stop=True,
                                )
                            for (s0, sw) in subs(pw):
                                nc.vector.tensor_mul(
                                    hbt[:, s0:s0 + sw], h1_t[:, s0:s0 + sw],
                                    sil[:, s0:s0 + sw]
                                )
                        hbts.append(hbt)
                        if pipe and hb == 0 and pending[0] is not None:
                            emit_o(pending[0])
                            pending[0] = None
                    if pipe:
                        pending[0] = (n, p0, pw, hbts, w_t)
                    else:
                        emit_o((n, p0, pw, hbts, w_t))

            def body():
                for n in range(N):
                    head(n)
                if pipe and pending[0] is not None:
                    emit_o(pending[0])
                    pending[0] = None

            if reps == 1:
                body()
            else:
                with tc.For_i(0, reps, 1):
                    body()
    nc.finalize()
    return nc


def _route(x, router_w):
    import jax
    import jax.numpy as jnp

    router_logits = jnp.asarray(x).reshape(B, N * D) @ jnp.asarray(router_w).T
    topk_logits, topk_idx = jax.lax.top_k(router_logits, 2)
    topk_w = jax.nn.softmax(topk_logits, axis=-1)
    return np.asarray(topk_idx), np.asarray(topk_w).astype(np.float32)


def _dispatch(x, topk_idx, topk_w):
    idx_list, wgt_list = [], []
    for e in range(E):
        sel = np.nonzero((topk_idx == e).any(axis=1))[0]
        we = np.where(topk_idx[sel, 0] == e, topk_w[sel, 0], topk_w[sel, 1])
        idx_list.append(sel)
        wgt_list.append(we.astype(np.float32))
    maxL = max(max(len(s) for s in idx_list), 1)
    C = ((maxL + 127) // 128) * 128
    chunks = []
    c0 = 0
    while c0 < C:
        cw = 512 if C - c0 >= 512 else C - c0
        chunks.append((c0, cw))
        c0 += cw
    return idx_list, wgt_list, C, tuple(chunks)


def _make_in_maps(x, w1, w_gate, w2, idx_list, wgt_list, C, bf16=USE_BF16):
    if bf16:
        import ml_dtypes
        dt = ml_dtypes.bfloat16
    else:
        dt = np.float32
    in_maps = []
    xTfull = np.ascontiguousarray(x.transpose(1, 2, 0).astype(dt))  # (N,128,B)
    for e in range(E):
        sel = idx_list[e]
        L = len(sel)
        xg = np.zeros((N, 128, C), dt)
        if L:
            xg[:, :, :L] = xTfull[:, :, sel]
        w2r = w2[e].reshape(N, HB, 128, 128).transpose(0, 2, 1, 3).reshape(N, 128, H)
        wcat = np.ascontiguousarray(np.concatenate(
            [w1[e].astype(dt), w_gate[e].astype(dt), w2r.astype(dt)], axis=2
        ))  # (N,128,3H)
        in_maps.append({"xT": xg, "wall": wcat})
    return in_maps


_runner_cache = {}


def _make_runner(nc):
    """Cached jitted executor equivalent to bass2jax.run_bass_via_pjrt,
    avoiding per-call retrace/rejit of the shard_map wrapper."""
    import jax
    import concourse.mybir as mybir
    from concourse import bass2jax
    from jax.sharding import Mesh, PartitionSpec
    from jax.experimental.shard_map import shard_map

    bass2jax.install_neuronx_cc_hook()
    partition_name = nc.partition_id_tensor.name if nc.partition_id_tensor else None
    in_names, out_names, out_avals, out_shapes = [], [], [], []
    for alloc in nc.m.functions[0].allocations:
        if not isinstance(alloc, mybir.MemoryLocationSet):
            continue
        name = alloc.memorylocations[0].name
        if alloc.kind == "ExternalInput":
            if name != partition_name:
                in_names.append(name)
        elif alloc.kind == "ExternalOutput":
            shape = tuple(alloc.tensor_shape)
            dtype = mybir.dt.np(alloc.dtype)
            out_names.append(name)
            out_avals.append(jax.core.ShapedArray(shape, dtype))
            out_shapes.append((shape, dtype))
    all_in_names = list(in_names) + list(out_names)
    if partition_name is not None:
        all_in_names.append(partition_name)

    def _body(*args):
        operands = list(args)
        if partition_name is not None:
            operands.append(bass2jax.partition_id_tensor())
        return tuple(bass2jax._bass_exec_p.bind(
            *operands,
            out_avals=tuple(out_avals),
            in_names=tuple(all_in_names),
            out_names=tuple(out_names),
            lowering_input_output_aliases=(),
            sim_require_finite=True,
            sim_require_nnan=True,
            nc=nc,
        ))

    mesh = Mesh(np.asarray(jax.devices()[:N_CORES]), ("core",))
    nio = len(in_names) + len(out_names)
    sharded = jax.jit(
        shard_map(_body, mesh=mesh,
                  in_specs=(PartitionSpec("core"),) * nio,
                  out_specs=(PartitionSpec("core"),) * len(out_names),
                  check_rep=False),
        keep_unused=True,
    )

    def run(in_maps):
        concat_in = [
            np.concatenate([np.asarray(in_maps[c][nm]) for c in range(N_CORES)],
                           axis=0)
            for nm in in_names
        ]
        concat_zeros = [
            np.zeros((N_CORES * s[0], *s[1:]), d) for (s, d) in out_shapes
        ]
        outs = sharded(*(concat_in + concat_zeros))
        outs = [np.asarray(o) for o in outs]
        results = []
        for c in range(N_CORES):
            res = {}
            for (nm, o, (s, d)) in zip(out_names, outs, out_shapes):
                res[nm] = o[c * s[0]:(c + 1) * s[0]]
            results.append(res)
        return results

    return run


def kernel(**inputs):
    x = np.asarray(inputs["x"], dtype=np.float32)
    router_w = np.asarray(inputs["router_w"], dtype=np.float32)
    w1 = np.asarray(inputs["w1"], dtype=np.float32)
    w_gate = np.asarray(inputs["w_gate"], dtype=np.float32)
    w2 = np.asarray(inputs["w2"], dtype=np.float32)

    topk_idx, topk_w = _route(x, router_w)
    idx_list, wgt_list, C, chunks = _dispatch(x, topk_idx, topk_w)

    key = (C, chunks, 1, USE_BF16)
    if key not in _nc_cache:
        _nc_cache[key] = _build_bass(C, chunks)
    nc = _nc_cache[key]

    in_maps = _make_in_maps(x, w1, w_gate, w2, idx_list, wgt_list, C)

    if key not in _runner_cache:
        from concourse import bass_utils
        res = bass_utils.run_bass_kernel_spmd(
            nc, in_maps, core_ids=list(range(N_CORES)), trace=False
        )
        results = res.results
        _runner_cache[key] = _make_runner(nc)
    else:
        results = _runner_cache[key](in_maps)

    out = np.zeros((B, N, D), np.float32)
    for e in range(E):
        sel = idx_list[e]
        L = len(sel)
        if L:
            yT = np.asarray(results[e]["yT"], dtype=np.float32)  # (N,128,C)
            out[sel] += yT[:, :, :L].transpose(2, 0, 1) * \
                wgt_list[e][:, None, None]
    return out

